# revision 12
# baseline (speedup 1.0000x reference)
"""AVSL-Graph fused kernel for Trainium2 (8 NeuronCores, data-parallel over batch).

Computation (per reference):
  for each level l in {0,1,2}:
    cam_l[b,r,hw] = sum_c w_l[r,c] * fmap_l[b,c,hw]          (1x1-conv GEMM)
    emb_l[b,r]    = mean_hw(cam_l) + bias_l[r]               (== pooled @ w.T + b)
    cert_l[b,r]   = std_hw(cam_l, ddof=1)
  link_l = sum_b  norm(pool2x2(cam_l))[b] @ norm(cam_{l+1})[b].T / B

Sharding: batch B=64 split 8 ways (8 samples/core). Each core computes its
embedding/certainty rows and an unnormalized link partial (sum over its local
samples, accumulated in PSUM); the host sums the 8 partials and divides by B.

The CAM GEMMs run with float32r operands (4-byte fp32 storage, reduced-precision
multiplier array at 4x the fp32 matmul rate; measured ~1.5e-4 rel err) with fp32
PSUM accumulation. Everything downstream (stats, norms, links) stays fp32.
"""

import os
import sys

import numpy as np

for _p in ("/opt/trn_rl_repo", "/root/.axon_site/_ro/trn_rl_repo"):
    if os.path.isdir(_p) and _p not in sys.path:
        sys.path.append(_p)

B = 64
R = 128
NCORES = 8
BLOC = B // NCORES  # 8 samples per core
CH = (512, 1024, 2048)
HW = (784, 196, 49)  # 28*28, 14*14, 7*7
KC = (4, 8, 16)  # contraction chunks of 128 per level

_CACHE = {}


def _build():
    import concourse.bass as bass
    import concourse.bacc as bacc
    import concourse.tile as tile
    from concourse import masks, mybir

    f32 = mybir.dt.float32
    f32r = mybir.dt.float32r
    AX = mybir.AxisListType
    AF = mybir.ActivationFunctionType

    nc = bacc.Bacc("TRN2", target_bir_lowering=False, debug=False)

    fm0 = nc.dram_tensor("fmap0", (BLOC, CH[0], HW[0]), f32r, kind="ExternalInput")
    # fmap1/fmap2 arrive host-transposed to (C, B_loc, HW) so HBM reads are
    # contiguous (b, hw) runs per (chunk, partition)
    fm1 = nc.dram_tensor("fmap1", (CH[1], BLOC, HW[1]), f32r, kind="ExternalInput")
    fm2 = nc.dram_tensor("fmap2", (CH[2], BLOC, HW[2]), f32r, kind="ExternalInput")
    wT = [
        nc.dram_tensor(f"w{l}T", (CH[l], R), f32r, kind="ExternalInput")
        for l in range(3)
    ]
    bias = nc.dram_tensor("bias", (R, 3), f32, kind="ExternalInput")
    emb_d = [
        nc.dram_tensor(f"emb{l}", (R, BLOC), f32, kind="ExternalOutput")
        for l in range(3)
    ]
    cert_d = [
        nc.dram_tensor(f"cert{l}", (R, BLOC), f32, kind="ExternalOutput")
        for l in range(3)
    ]
    link_d = [
        nc.dram_tensor(f"link{l}", (R, R), f32, kind="ExternalOutput")
        for l in range(2)
    ]

    with tile.TileContext(nc) as tc:
        with (
            tc.tile_pool(name="const", bufs=1) as const,
            tc.tile_pool(name="f0p", bufs=3) as f0p,
            tc.tile_pool(name="f1p", bufs=2) as f1p,
            tc.tile_pool(name="f2p", bufs=1) as f2p,
            tc.tile_pool(name="work", bufs=3) as work,
            tc.tile_pool(name="stats", bufs=3) as stats,
            tc.tile_pool(name="outp", bufs=1) as outp,
            tc.tile_pool(name="ps_cam0", bufs=1, space="PSUM") as ps_cam0,
            tc.tile_pool(name="ps_cam1", bufs=1, space="PSUM") as ps_cam1,
            tc.tile_pool(name="ps_cam2", bufs=1, space="PSUM") as ps_cam2,
            tc.tile_pool(name="ps_link", bufs=1, space="PSUM") as ps_link,
            tc.tile_pool(name="ps_tr", bufs=2, space="PSUM") as ps_tr,
        ):
            # ---- constants (weights cast to f32r during DMA for the GEMMs) ----
            wt = []
            for l in range(3):
                t = const.tile([128, KC[l], 128], f32r, tag=f"wt{l}", name=f"wt{l}")
                nc.sync.dma_start(
                    t[:], wT[l][:].rearrange("(k p) r -> p k r", p=128)
                )
                wt.append(t)
            bias_t = const.tile([128, 3], f32, tag="bias")
            nc.sync.dma_start(bias_t[:], bias[:])
            ident = const.tile([128, 128], f32, tag="ident")
            masks.make_identity(nc, ident[:])

            # ---- output accumulators in SBUF ----
            emb_t = [outp.tile([128, BLOC], f32, tag=f"emb{l}", name=f"emb_t{l}")
                     for l in range(3)]
            cert_t = [outp.tile([128, BLOC], f32, tag=f"cert{l}", name=f"cert_t{l}")
                      for l in range(3)]
            # transposed normalized pooled cam1, one slot per sample (for link1)
            loT1s = outp.tile([128, BLOC, 128], f32, tag="loT1s")

            f2t = f2p.tile([128, KC[2], BLOC, HW[2]], f32r, tag="f2")
            fm2v = fm2[:].rearrange("(k p) b hw -> p k b hw", p=128)

            link0_t = ps_link.tile([128, 128], f32, tag="link0")
            link1_t = ps_link.tile([128, 128], f32, tag="link1")
            cam2 = ps_cam2.tile([128, BLOC, HW[2]], f32, tag="cam2")

            # ============ phase A: levels 0+1, link0, save loT1 ============
            cam1 = None
            for b in range(BLOC):
                if b % 2 == 0:
                    f1t = f1p.tile([128, KC[1], 2, HW[1]], f32r, tag="f1")
                    nc.sync.dma_start(
                        f1t[:],
                        fm1[:, b : b + 2, :].rearrange(
                            "(k p) b hw -> p k b hw", p=128
                        ),
                    )
                    cam1 = ps_cam1.tile([128, 2, HW[1]], f32, tag="cam1")
                    for k in range(KC[1]):
                        nc.tensor.matmul(
                            cam1[:],
                            wt[1][:, k, :],
                            f1t[:, k, :, :],
                            start=(k == 0),
                            stop=(k == KC[1] - 1),
                        )

                f0t = f0p.tile([128, KC[0], HW[0]], f32r, tag="f0")
                nc.sync.dma_start(
                    f0t[:], fm0[b].rearrange("(k p) hw -> p k hw", p=128)
                )
                # stream fmap2 in the background, 4 contraction chunks every
                # other iteration, and fold its GEMM chunks into the loop so
                # cam2 is ready right after the last iteration
                if b % 2 == 0:
                    g = b // 2
                    nc.sync.dma_start(
                        f2t[:, 4 * g : 4 * g + 4, :, :],
                        fm2v[:, 4 * g : 4 * g + 4, :, :],
                    )

                cam0 = ps_cam0.tile([128, 2, 512], f32, tag="cam0")
                for k in range(KC[0]):
                    for h in range(2):
                        nc.tensor.matmul(
                            cam0[:, h, 0:392],
                            wt[0][:, k, :],
                            f0t[:, k, h * 392 : (h + 1) * 392],
                            start=(k == 0),
                            stop=(k == KC[0] - 1),
                        )
                # two cam2 contraction chunks per iteration (group spans phase A)
                for k in (2 * b, 2 * b + 1):
                    nc.tensor.matmul(
                        cam2[:],
                        wt[2][:, k, :],
                        f2t[:, k, :, :],
                        start=(k == 0),
                        stop=(k == KC[2] - 1),
                    )

                # ---- level 0 stats + pooled lo0 ----
                st0 = stats.tile([128, 2, 6], f32, tag="st0")
                for h in range(2):
                    nc.vector.bn_stats(st0[:, h, :], cam0[:, h, 0:392])
                mv0 = stats.tile([128, 2], f32, tag="mv0")
                nc.vector.bn_aggr(mv0[:], st0[:])
                nc.scalar.activation(
                    emb_t[0][:, b : b + 1], mv0[:, 0:1], AF.Identity,
                    bias=bias_t[:, 0:1], scale=1.0,
                )
                nc.scalar.activation(
                    cert_t[0][:, b : b + 1], mv0[:, 1:2], AF.Sqrt,
                    scale=float(HW[0]) / float(HW[0] - 1),
                )
                # 2x2 sum-pool (the 1/4 factor cancels in the normalization;
                # norms are O(1..100) >> the 1e-12 eps clamp, so it is omitted)
                praw0 = work.tile([128, 196], f32, tag="praw0")
                for h in range(2):
                    nc.vector.reduce_sum(
                        praw0[:, h * 98 : (h + 1) * 98].rearrange(
                            "p (oh ow) -> p oh ow", oh=7
                        ),
                        cam0[:, h, 0:392].rearrange(
                            "p (oh dh ow dw) -> p oh ow dh dw", oh=7, dh=2, ow=14, dw=2
                        ),
                        axis=AX.XY,
                    )
                sq0 = work.tile([128, 196], f32, tag="sq0")
                q0 = stats.tile([128, 1], f32, tag="q0")
                nc.scalar.activation(sq0[:], praw0[:], AF.Square, accum_out=q0[:])
                inv_lo0 = stats.tile([128, 1], f32, tag="inv_lo0")
                nc.scalar.sqrt(inv_lo0[:], q0[:])
                nc.vector.reciprocal(inv_lo0[:], inv_lo0[:])
                lo0 = work.tile([128, 196], f32, tag="lo0")
                nc.vector.tensor_scalar_mul(lo0[:], praw0[:], inv_lo0[:])

                tr0 = ps_tr.tile([128, 4, 128], f32, tag="tr")
                nc.tensor.transpose(tr0[:, 0, :], lo0[:, 0:128], ident[:])
                nc.tensor.transpose(tr0[0:68, 1, :], lo0[:, 128:196], ident[:])

                # ---- level 1 stats + hi1 + pooled lo1 ----
                camb = cam1[:, b % 2, :]
                st1 = stats.tile([128, 6], f32, tag="st1")
                nc.vector.bn_stats(st1[:], camb)
                mv1 = stats.tile([128, 2], f32, tag="mv1")
                nc.vector.bn_aggr(mv1[:], st1[:])
                nc.scalar.activation(
                    emb_t[1][:, b : b + 1], mv1[:, 0:1], AF.Identity,
                    bias=bias_t[:, 1:2], scale=1.0,
                )
                nc.scalar.activation(
                    cert_t[1][:, b : b + 1], mv1[:, 1:2], AF.Sqrt,
                    scale=float(HW[1]) / float(HW[1] - 1),
                )
                # ||cam1|| = sqrt(n*(var + mean^2))
                nh1 = stats.tile([128, 1], f32, tag="nh1")
                nc.gpsimd.tensor_mul(nh1[:], mv1[:, 0:1], mv1[:, 0:1])
                nc.gpsimd.tensor_add(nh1[:], nh1[:], mv1[:, 1:2])
                nc.scalar.activation(nh1[:], nh1[:], AF.Sqrt, scale=float(HW[1]))
                nc.vector.reciprocal(nh1[:], nh1[:])
                hi1 = work.tile([128, 196], f32, tag="hi1")
                nc.vector.tensor_scalar_mul(hi1[:], camb, nh1[:])
                nc.tensor.transpose(tr0[:, 2, :], hi1[:, 0:128], ident[:])
                nc.tensor.transpose(tr0[0:68, 3, :], hi1[:, 128:196], ident[:])

                praw1 = work.tile([128, 49], f32, tag="praw1")
                nc.vector.reduce_sum(
                    praw1[:].rearrange("p (oh ow) -> p oh ow", oh=7),
                    camb.rearrange(
                        "p (oh dh ow dw) -> p oh ow dh dw", oh=7, dh=2, ow=7, dw=2
                    ),
                    axis=AX.XY,
                )
                sq1 = work.tile([128, 49], f32, tag="sq1")
                q1 = stats.tile([128, 1], f32, tag="q1")
                nc.scalar.activation(sq1[:], praw1[:], AF.Square, accum_out=q1[:])
                inv_lo1 = stats.tile([128, 1], f32, tag="inv_lo1")
                nc.scalar.sqrt(inv_lo1[:], q1[:])
                nc.vector.reciprocal(inv_lo1[:], inv_lo1[:])
                lo1 = work.tile([128, 49], f32, tag="lo1")
                nc.vector.tensor_scalar_mul(lo1[:], praw1[:], inv_lo1[:])

                tr1 = ps_tr.tile([128, 4, 128], f32, tag="tr")
                nc.tensor.transpose(tr1[0:49, 0, :], lo1[:], ident[:])

                # ---- PSUM -> SBUF: one merged copy for the 4 tr0 slots ----
                loHiT = work.tile([128, 4, 128], f32, tag="loHiT")
                nc.vector.tensor_copy(loHiT[:], tr0[:])
                nc.vector.tensor_copy(loT1s[:, b, :], tr1[:, 0, :])

                # ---- link0 partial accumulation ----
                nc.tensor.matmul(
                    link0_t[:], loHiT[:, 0, :], loHiT[:, 2, :],
                    start=(b == 0), stop=False,
                )
                nc.tensor.matmul(
                    link0_t[:], loHiT[0:68, 1, :], loHiT[0:68, 3, :],
                    start=False, stop=(b == BLOC - 1),
                )

            # ============ phase B: level 2 stats + link1 ============
            sqa = work.tile([128, BLOC, HW[2]], f32, tag="sqa")
            nc.scalar.activation(sqa[:], cam2[:], AF.Square)
            S2 = stats.tile([128, BLOC], f32, tag="S2")
            nc.vector.reduce_sum(S2[:], cam2[:], axis=AX.X)
            Q2 = stats.tile([128, BLOC], f32, tag="Q2")
            nc.vector.reduce_sum(Q2[:], sqa[:], axis=AX.X)
            nc.vector.tensor_scalar(
                emb_t[2][:], S2[:], 1.0 / HW[2], bias_t[:, 2:3],
                op0=mybir.AluOpType.mult, op1=mybir.AluOpType.add,
            )
            v2a = stats.tile([128, BLOC], f32, tag="v2a")
            nc.gpsimd.tensor_mul(v2a[:], S2[:], S2[:])
            nc.gpsimd.tensor_scalar_mul(v2a[:], v2a[:], -1.0 / HW[2])
            nc.gpsimd.tensor_add(v2a[:], v2a[:], Q2[:])
            nc.scalar.activation(
                cert_t[2][:], v2a[:], AF.Sqrt, scale=1.0 / (HW[2] - 1)
            )
            nh2a = stats.tile([128, BLOC], f32, tag="nh2a")
            nc.scalar.sqrt(nh2a[:], Q2[:])
            nc.vector.reciprocal(nh2a[:], nh2a[:])
            hi2a = work.tile([128, BLOC, HW[2]], f32, tag="hi2a")
            nh2v = nh2a[:]
            nh2b = bass.AP(
                tensor=nh2v.tensor, offset=nh2v.offset,
                ap=[nh2v.ap[0], nh2v.ap[1], [0, HW[2]]],
            )
            nc.vector.tensor_mul(hi2a[:], cam2[:], nh2b)

            hiT2s = []
            for g in range(2):
                trg = ps_tr.tile([128, 4, 128], f32, tag="tr", name=f"tr2_{g}")
                for j in range(4):
                    nc.tensor.transpose(
                        trg[0:49, j, :], hi2a[:, 4 * g + j, :], ident[:]
                    )
                ht = work.tile([128, 4, 128], f32, tag="hiT2s", name=f"hiT2s{g}")
                nc.vector.tensor_copy(ht[:], trg[:])
                hiT2s.append(ht)
            for b in range(BLOC):
                nc.tensor.matmul(
                    link1_t[:], loT1s[0:49, b, :], hiT2s[b // 4][0:49, b % 4, :],
                    start=(b == 0), stop=(b == BLOC - 1),
                )

            # ---- write outputs ----
            link_sb = outp.tile([128, 2, 128], f32, tag="link_sb")
            nc.vector.tensor_copy(link_sb[:, 0, :], link0_t[:])
            nc.vector.tensor_copy(link_sb[:, 1, :], link1_t[:])
            nc.sync.dma_start(link_d[0][:], link_sb[:, 0, :])
            nc.sync.dma_start(link_d[1][:], link_sb[:, 1, :])
            for l in range(3):
                nc.sync.dma_start(emb_d[l][:], emb_t[l][:])
                nc.sync.dma_start(cert_d[l][:], cert_t[l][:])

    nc.compile()
    return nc


def _get_nc():
    if "nc" not in _CACHE:
        _CACHE["nc"] = _build()
    return _CACHE["nc"]


def kernel(**inputs):
    from concourse.bass_utils import run_bass_kernel_spmd

    fmaps = [np.asarray(inputs[f"fmap{l}"], dtype=np.float32) for l in range(3)]
    ws = [np.asarray(inputs[f"w{l}"], dtype=np.float32) for l in range(3)]
    bs = [np.asarray(inputs[f"b{l}"], dtype=np.float32) for l in range(3)]

    wTs = [np.ascontiguousarray(w.T) for w in ws]
    bias = np.ascontiguousarray(np.stack(bs, axis=1))  # (R, 3)

    f0 = np.ascontiguousarray(fmaps[0].reshape(B, CH[0], HW[0]))
    # fmap1/fmap2 shipped channel-major per core: (C, B_loc, HW)
    f1 = np.ascontiguousarray(
        fmaps[1].reshape(B, CH[1], HW[1]).transpose(1, 0, 2)
    )
    f2 = np.ascontiguousarray(
        fmaps[2].reshape(B, CH[2], HW[2]).transpose(1, 0, 2)
    )

    nc = _get_nc()
    in_maps = []
    for i in range(NCORES):
        sl = slice(i * BLOC, (i + 1) * BLOC)
        m = {
            "bias": bias,
            "w0T": wTs[0], "w1T": wTs[1], "w2T": wTs[2],
            "fmap0": f0[sl],
            "fmap1": np.ascontiguousarray(f1[:, sl, :]),
            "fmap2": np.ascontiguousarray(f2[:, sl, :]),
        }
        in_maps.append(m)

    trace = os.environ.get("BASS_KERNEL_TRACE") == "1"
    kw = {}
    if trace and _CACHE.get("tmpdir"):
        kw["tmpdir"] = _CACHE["tmpdir"]
    res = run_bass_kernel_spmd(nc, in_maps, core_ids=list(range(NCORES)),
                               trace=trace, **kw)
    _CACHE["last_result"] = res
    rs = res.results

    embs = [
        np.concatenate([rs[i][f"emb{l}"].T for i in range(NCORES)], axis=0)
        for l in range(3)
    ]
    certs = [
        np.concatenate([rs[i][f"cert{l}"].T for i in range(NCORES)], axis=0)
        for l in range(3)
    ]
    links = [
        (
            np.sum(
                np.stack([rs[i][f"link{l}"] for i in range(NCORES)]).astype(np.float64),
                axis=0,
            )
            / B
        ).astype(np.float32)
        for l in range(2)
    ]
    return (*embs, *certs, *links)


# revision 13
# speedup vs baseline: 1.2203x; 1.2203x over previous
"""AVSL-Graph fused kernel for Trainium2 (8 NeuronCores, data-parallel over batch).

Computation (per reference):
  for each level l in {0,1,2}:
    cam_l[b,r,hw] = sum_c w_l[r,c] * fmap_l[b,c,hw]          (1x1-conv GEMM)
    emb_l[b,r]    = mean_hw(cam_l) + bias_l[r]               (== pooled @ w.T + b)
    cert_l[b,r]   = std_hw(cam_l, ddof=1)
  link_l = sum_b  norm(pool2x2(cam_l))[b] @ norm(cam_{l+1})[b].T / B

Sharding: batch B=64 split 8 ways (8 samples/core). Each core computes its
embedding/certainty rows and an unnormalized link partial (sum over its local
samples, accumulated in PSUM); the host sums the 8 partials and divides by B.

The CAM GEMMs run with float32r operands (4-byte fp32 storage, reduced-precision
multiplier array at 4x the fp32 matmul rate; measured ~1.5e-4 rel err) with fp32
PSUM accumulation. Everything downstream (stats, norms, links) stays fp32.
"""

import os
import sys

import numpy as np

for _p in ("/opt/trn_rl_repo", "/root/.axon_site/_ro/trn_rl_repo"):
    if os.path.isdir(_p) and _p not in sys.path:
        sys.path.append(_p)

B = 64
R = 128
NCORES = 8
BLOC = B // NCORES  # 8 samples per core
CH = (512, 1024, 2048)
HW = (784, 196, 49)  # 28*28, 14*14, 7*7
KC = (4, 8, 16)  # contraction chunks of 128 per level

_CACHE = {}


def _build():
    import concourse.bass as bass
    import concourse.bacc as bacc
    import concourse.tile as tile
    from concourse import masks, mybir

    f32 = mybir.dt.float32
    f32r = mybir.dt.float32r
    AX = mybir.AxisListType
    AF = mybir.ActivationFunctionType

    nc = bacc.Bacc("TRN2", target_bir_lowering=False, debug=False)

    fm0 = nc.dram_tensor("fmap0", (BLOC, CH[0], HW[0]), f32r, kind="ExternalInput")
    # fmap1/fmap2 arrive host-transposed to (C, B_loc, HW) so HBM reads are
    # contiguous (b, hw) runs per (chunk, partition)
    fm1 = nc.dram_tensor("fmap1", (CH[1], BLOC, HW[1]), f32r, kind="ExternalInput")
    fm2 = nc.dram_tensor("fmap2", (CH[2], BLOC, HW[2]), f32r, kind="ExternalInput")
    wT = [
        nc.dram_tensor(f"w{l}T", (CH[l], R), f32r, kind="ExternalInput")
        for l in range(3)
    ]
    bias = nc.dram_tensor("bias", (R, 3), f32, kind="ExternalInput")
    emb_d = [
        nc.dram_tensor(f"emb{l}", (R, BLOC), f32, kind="ExternalOutput")
        for l in range(3)
    ]
    cert_d = [
        nc.dram_tensor(f"cert{l}", (R, BLOC), f32, kind="ExternalOutput")
        for l in range(3)
    ]
    link_d = [
        nc.dram_tensor(f"link{l}", (R, R), f32, kind="ExternalOutput")
        for l in range(2)
    ]

    with tile.TileContext(nc) as tc:
        with (
            tc.tile_pool(name="const", bufs=1) as const,
            tc.tile_pool(name="f0p", bufs=3) as f0p,
            tc.tile_pool(name="f1p", bufs=2) as f1p,
            tc.tile_pool(name="f2p", bufs=1) as f2p,
            tc.tile_pool(name="work", bufs=3) as work,
            tc.tile_pool(name="stats", bufs=3) as stats,
            tc.tile_pool(name="outp", bufs=1) as outp,
            tc.tile_pool(name="ps_cam0", bufs=1, space="PSUM") as ps_cam0,
            tc.tile_pool(name="ps_cam1", bufs=1, space="PSUM") as ps_cam1,
            tc.tile_pool(name="ps_cam2", bufs=1, space="PSUM") as ps_cam2,
            tc.tile_pool(name="ps_link", bufs=1, space="PSUM") as ps_link,
            tc.tile_pool(name="ps_tr", bufs=2, space="PSUM") as ps_tr,
        ):
            # ---- constants ----
            wt = []
            for l in range(3):
                t = const.tile([128, KC[l], 128], f32r, tag=f"wt{l}", name=f"wt{l}")
                nc.sync.dma_start(
                    t[:], wT[l][:].rearrange("(k p) r -> p k r", p=128)
                )
                wt.append(t)
            bias_t = const.tile([128, 3], f32, tag="bias")
            nc.sync.dma_start(bias_t[:], bias[:])
            ident = const.tile([128, 128], f32, tag="ident")
            masks.make_identity(nc, ident[:])

            # ---- output accumulators in SBUF ----
            emb_t = [outp.tile([128, BLOC], f32, tag=f"emb{l}", name=f"emb_t{l}")
                     for l in range(3)]
            cert_t = [outp.tile([128, BLOC], f32, tag=f"cert{l}", name=f"cert_t{l}")
                      for l in range(3)]
            loT1s = outp.tile([128, BLOC, 128], f32, tag="loT1s")

            f2t = f2p.tile([128, KC[2], BLOC, HW[2]], f32r, tag="f2")
            fm2v = fm2[:].rearrange("(k p) b hw -> p k b hw", p=128)

            link0_t = ps_link.tile([128, 128], f32, tag="link0")
            link1_t = ps_link.tile([128, 128], f32, tag="link1")
            cam2 = ps_cam2.tile([128, BLOC, HW[2]], f32, tag="cam2")

            def stage1(b):
                """DMAs, GEMMs, evacuation, stats, normalized lo/hi for sample b."""
                nonlocal cam1sb
                if b % 2 == 0:
                    f1t = f1p.tile([128, KC[1], 2, HW[1]], f32r, tag="f1",
                                   name=f"f1t_{b}")
                    nc.sync.dma_start(
                        f1t[:],
                        fm1[:, b : b + 2, :].rearrange(
                            "(k p) b hw -> p k b hw", p=128
                        ),
                    )
                    cam1 = ps_cam1.tile([128, 2, HW[1]], f32, tag="cam1",
                                        name=f"cam1_{b}")
                    for k in range(KC[1]):
                        nc.tensor.matmul(
                            cam1[:], wt[1][:, k, :], f1t[:, k, :, :],
                            start=(k == 0), stop=(k == KC[1] - 1),
                        )
                f0t = f0p.tile([128, KC[0], HW[0]], f32r, tag="f0", name=f"f0t_{b}")
                nc.sync.dma_start(
                    f0t[:], fm0[b].rearrange("(k p) hw -> p k hw", p=128)
                )
                if b % 2 == 0:
                    g = b // 2
                    nc.sync.dma_start(
                        f2t[:, 4 * g : 4 * g + 4, :, :],
                        fm2v[:, 4 * g : 4 * g + 4, :, :],
                    )
                cam0 = ps_cam0.tile([128, 2, 512], f32, tag="cam0",
                                    name=f"cam0_{b}")
                for k in range(KC[0]):
                    for h in range(2):
                        nc.tensor.matmul(
                            cam0[:, h, 0:392],
                            wt[0][:, k, :],
                            f0t[:, k, h * 392 : (h + 1) * 392],
                            start=(k == 0), stop=(k == KC[0] - 1),
                        )
                # cam2 contraction chunks, 2 per iteration, delayed 2 iterations
                # behind their DMA so the PE never waits on fmap2
                if b >= 2:
                    for k in (2 * b - 4, 2 * b - 3):
                        nc.tensor.matmul(
                            cam2[:], wt[2][:, k, :], f2t[:, k, :, :],
                            start=(k == 0), stop=(k == KC[2] - 1),
                        )

                # ---- evacuate PSUM fast (frees the banks for the next GEMM) ----
                cam0sb = work.tile([128, 2, 392], f32, tag="cam0sb",
                                   name=f"cam0sb_{b}")
                nc.vector.tensor_copy(cam0sb[:], cam0[:, :, 0:392])
                if b % 2 == 0:
                    cam1sb = work.tile([128, 2, HW[1]], f32, tag="cam1sb",
                                       name=f"cam1sb_{b}")
                    nc.vector.tensor_copy(cam1sb[:], cam1[:])

                # ---- level 0 stats ----
                st0 = stats.tile([128, 2, 6], f32, tag="st0", name=f"st0_{b}")
                for h in range(2):
                    nc.vector.bn_stats(st0[:, h, :], cam0sb[:, h, :])
                mv0 = stats.tile([128, 2], f32, tag="mv0", name=f"mv0_{b}")
                nc.vector.bn_aggr(mv0[:], st0[:])
                nc.scalar.activation(
                    emb_t[0][:, b : b + 1], mv0[:, 0:1], AF.Identity,
                    bias=bias_t[:, 0:1], scale=1.0,
                )
                nc.scalar.activation(
                    cert_t[0][:, b : b + 1], mv0[:, 1:2], AF.Sqrt,
                    scale=float(HW[0]) / float(HW[0] - 1),
                )
                # ---- 2x2 sum-pool of cam0 on gpsimd (scale cancels in norm) ----
                praw0 = work.tile([128, 196], f32, tag="praw0", name=f"praw0_{b}")
                tp = work.tile([128, 2, 98], f32, tag="pool_tmp", name=f"ptmp_{b}")
                for h in range(2):
                    v = cam0sb[:, h, :].rearrange(
                        "p (oh dh ow dw) -> p oh ow dh dw", oh=7, dh=2, ow=14, dw=2
                    )
                    t1 = tp[:, 0, :].rearrange("p (oh ow) -> p oh ow", oh=7)
                    t2 = tp[:, 1, :].rearrange("p (oh ow) -> p oh ow", oh=7)
                    po = praw0[:, h * 98 : (h + 1) * 98].rearrange(
                        "p (oh ow) -> p oh ow", oh=7
                    )
                    nc.gpsimd.tensor_add(t1, v[:, :, :, 0, 0], v[:, :, :, 0, 1])
                    nc.gpsimd.tensor_add(t2, v[:, :, :, 1, 0], v[:, :, :, 1, 1])
                    nc.gpsimd.tensor_add(po, t1, t2)
                sq0 = work.tile([128, 196], f32, tag="sq0", name=f"sq0_{b}")
                q0 = stats.tile([128, 1], f32, tag="q0", name=f"q0_{b}")
                nc.scalar.activation(sq0[:], praw0[:], AF.Square, accum_out=q0[:])
                inv_lo0 = stats.tile([128, 1], f32, tag="inv_lo0", name=f"ilo0_{b}")
                nc.scalar.sqrt(inv_lo0[:], q0[:])
                nc.vector.reciprocal(inv_lo0[:], inv_lo0[:])
                lo0 = work.tile([128, 196], f32, tag="lo0", name=f"lo0_{b}")
                nc.vector.tensor_scalar_mul(lo0[:], praw0[:], inv_lo0[:])

                # ---- level 1 stats + hi1 + pooled lo1 ----
                camb = cam1sb[:, b % 2, :]
                st1 = stats.tile([128, 6], f32, tag="st1", name=f"st1_{b}")
                nc.vector.bn_stats(st1[:], camb)
                mv1 = stats.tile([128, 2], f32, tag="mv1", name=f"mv1_{b}")
                nc.vector.bn_aggr(mv1[:], st1[:])
                nc.scalar.activation(
                    emb_t[1][:, b : b + 1], mv1[:, 0:1], AF.Identity,
                    bias=bias_t[:, 1:2], scale=1.0,
                )
                nc.scalar.activation(
                    cert_t[1][:, b : b + 1], mv1[:, 1:2], AF.Sqrt,
                    scale=float(HW[1]) / float(HW[1] - 1),
                )
                nh1 = stats.tile([128, 1], f32, tag="nh1", name=f"nh1_{b}")
                nc.gpsimd.tensor_mul(nh1[:], mv1[:, 0:1], mv1[:, 0:1])
                nc.gpsimd.tensor_add(nh1[:], nh1[:], mv1[:, 1:2])
                nc.scalar.activation(nh1[:], nh1[:], AF.Sqrt, scale=float(HW[1]))
                nc.vector.reciprocal(nh1[:], nh1[:])
                hi1 = work.tile([128, 196], f32, tag="hi1", name=f"hi1_{b}")
                nc.vector.tensor_scalar_mul(hi1[:], camb, nh1[:])

                praw1 = work.tile([128, 49], f32, tag="praw1", name=f"praw1_{b}")
                v1 = camb.rearrange(
                    "p (oh dh ow dw) -> p oh ow dh dw", oh=7, dh=2, ow=7, dw=2
                )
                tq = work.tile([128, 2, 49], f32, tag="pool_tmp1", name=f"ptmp1_{b}")
                u1 = tq[:, 0, :].rearrange("p (oh ow) -> p oh ow", oh=7)
                u2 = tq[:, 1, :].rearrange("p (oh ow) -> p oh ow", oh=7)
                pv = praw1[:].rearrange("p (oh ow) -> p oh ow", oh=7)
                nc.gpsimd.tensor_add(u1, v1[:, :, :, 0, 0], v1[:, :, :, 0, 1])
                nc.gpsimd.tensor_add(u2, v1[:, :, :, 1, 0], v1[:, :, :, 1, 1])
                nc.gpsimd.tensor_add(pv, u1, u2)
                sq1 = work.tile([128, 49], f32, tag="sq1", name=f"sq1_{b}")
                q1 = stats.tile([128, 1], f32, tag="q1", name=f"q1_{b}")
                nc.scalar.activation(sq1[:], praw1[:], AF.Square, accum_out=q1[:])
                inv_lo1 = stats.tile([128, 1], f32, tag="inv_lo1", name=f"ilo1_{b}")
                nc.scalar.sqrt(inv_lo1[:], q1[:])
                nc.vector.reciprocal(inv_lo1[:], inv_lo1[:])
                lo1 = work.tile([128, 49], f32, tag="lo1", name=f"lo1_{b}")
                nc.vector.tensor_scalar_mul(lo1[:], praw1[:], inv_lo1[:])
                return lo0, hi1, lo1

            def stage2(b, lo0, hi1, lo1):
                """Transposes + link0 accumulation for sample b (runs 1 behind)."""
                tr0 = ps_tr.tile([128, 4, 128], f32, tag="tr", name=f"tr0_{b}")
                nc.tensor.transpose(tr0[:, 0, :], lo0[:, 0:128], ident[:])
                nc.tensor.transpose(tr0[0:68, 1, :], lo0[:, 128:196], ident[:])
                nc.tensor.transpose(tr0[:, 2, :], hi1[:, 0:128], ident[:])
                nc.tensor.transpose(tr0[0:68, 3, :], hi1[:, 128:196], ident[:])
                tr1 = ps_tr.tile([128, 4, 128], f32, tag="tr", name=f"tr1_{b}")
                nc.tensor.transpose(tr1[0:49, 0, :], lo1[:], ident[:])
                loHiT = work.tile([128, 4, 128], f32, tag="loHiT", name=f"loHiT_{b}")
                nc.vector.tensor_copy(loHiT[:], tr0[:])
                nc.vector.tensor_copy(loT1s[:, b, :], tr1[:, 0, :])
                nc.tensor.matmul(
                    link0_t[:], loHiT[:, 0, :], loHiT[:, 2, :],
                    start=(b == 0), stop=False,
                )
                nc.tensor.matmul(
                    link0_t[:], loHiT[0:68, 1, :], loHiT[0:68, 3, :],
                    start=False, stop=(b == BLOC - 1),
                )

            # ============ phase A: pipelined over samples ============
            cam1sb = None
            prev = None
            for b in range(BLOC + 1):
                if b >= 1:
                    stage2(b - 1, *prev)
                if b < BLOC:
                    prev = stage1(b)

            # last cam2 contraction chunks
            for k in (KC[2] - 4, KC[2] - 3, KC[2] - 2, KC[2] - 1):
                nc.tensor.matmul(
                    cam2[:], wt[2][:, k, :], f2t[:, k, :, :],
                    start=(k == 0), stop=(k == KC[2] - 1),
                )

            # ============ phase B: level 2 stats + link1 (batched) ============
            sqa = work.tile([128, BLOC, HW[2]], f32, tag="sqa")
            nc.scalar.activation(sqa[:], cam2[:], AF.Square)
            S2 = stats.tile([128, BLOC], f32, tag="S2")
            nc.vector.reduce_sum(S2[:], cam2[:], axis=AX.X)
            Q2 = stats.tile([128, BLOC], f32, tag="Q2")
            nc.vector.reduce_sum(Q2[:], sqa[:], axis=AX.X)
            nc.vector.tensor_scalar(
                emb_t[2][:], S2[:], 1.0 / HW[2], bias_t[:, 2:3],
                op0=mybir.AluOpType.mult, op1=mybir.AluOpType.add,
            )
            v2a = stats.tile([128, BLOC], f32, tag="v2a")
            nc.gpsimd.tensor_mul(v2a[:], S2[:], S2[:])
            nc.gpsimd.tensor_scalar_mul(v2a[:], v2a[:], -1.0 / HW[2])
            nc.gpsimd.tensor_add(v2a[:], v2a[:], Q2[:])
            nc.scalar.activation(
                cert_t[2][:], v2a[:], AF.Sqrt, scale=1.0 / (HW[2] - 1)
            )
            nh2a = stats.tile([128, BLOC], f32, tag="nh2a")
            nc.scalar.sqrt(nh2a[:], Q2[:])
            nc.vector.reciprocal(nh2a[:], nh2a[:])
            hi2a = work.tile([128, BLOC, HW[2]], f32, tag="hi2a")
            nh2v = nh2a[:]
            nh2b = bass.AP(
                tensor=nh2v.tensor, offset=nh2v.offset,
                ap=[nh2v.ap[0], nh2v.ap[1], [0, HW[2]]],
            )
            nc.vector.tensor_mul(hi2a[:], cam2[:], nh2b)

            hiT2s = []
            for g in range(2):
                trg = ps_tr.tile([128, 4, 128], f32, tag="tr", name=f"tr2_{g}")
                for j in range(4):
                    nc.tensor.transpose(
                        trg[0:49, j, :], hi2a[:, 4 * g + j, :], ident[:]
                    )
                ht = work.tile([128, 4, 128], f32, tag="hiT2s", name=f"hiT2s{g}")
                nc.vector.tensor_copy(ht[:], trg[:])
                hiT2s.append(ht)
            for b in range(BLOC):
                nc.tensor.matmul(
                    link1_t[:], loT1s[0:49, b, :], hiT2s[b // 4][0:49, b % 4, :],
                    start=(b == 0), stop=(b == BLOC - 1),
                )

            # ---- write outputs ----
            link_sb = outp.tile([128, 2, 128], f32, tag="link_sb")
            nc.vector.tensor_copy(link_sb[:, 0, :], link0_t[:])
            nc.vector.tensor_copy(link_sb[:, 1, :], link1_t[:])
            nc.sync.dma_start(link_d[0][:], link_sb[:, 0, :])
            nc.sync.dma_start(link_d[1][:], link_sb[:, 1, :])
            for l in range(3):
                nc.sync.dma_start(emb_d[l][:], emb_t[l][:])
                nc.sync.dma_start(cert_d[l][:], cert_t[l][:])

    nc.compile()
    return nc


def _get_nc():
    if "nc" not in _CACHE:
        _CACHE["nc"] = _build()
    return _CACHE["nc"]


def kernel(**inputs):
    from concourse.bass_utils import run_bass_kernel_spmd

    fmaps = [np.asarray(inputs[f"fmap{l}"], dtype=np.float32) for l in range(3)]
    ws = [np.asarray(inputs[f"w{l}"], dtype=np.float32) for l in range(3)]
    bs = [np.asarray(inputs[f"b{l}"], dtype=np.float32) for l in range(3)]

    wTs = [np.ascontiguousarray(w.T) for w in ws]
    bias = np.ascontiguousarray(np.stack(bs, axis=1))  # (R, 3)

    f0 = np.ascontiguousarray(fmaps[0].reshape(B, CH[0], HW[0]))
    # fmap1/fmap2 shipped channel-major per core: (C, B_loc, HW)
    f1 = np.ascontiguousarray(
        fmaps[1].reshape(B, CH[1], HW[1]).transpose(1, 0, 2)
    )
    f2 = np.ascontiguousarray(
        fmaps[2].reshape(B, CH[2], HW[2]).transpose(1, 0, 2)
    )

    nc = _get_nc()
    in_maps = []
    for i in range(NCORES):
        sl = slice(i * BLOC, (i + 1) * BLOC)
        m = {
            "bias": bias,
            "w0T": wTs[0], "w1T": wTs[1], "w2T": wTs[2],
            "fmap0": f0[sl],
            "fmap1": np.ascontiguousarray(f1[:, sl, :]),
            "fmap2": np.ascontiguousarray(f2[:, sl, :]),
        }
        in_maps.append(m)

    trace = os.environ.get("BASS_KERNEL_TRACE") == "1"
    kw = {}
    if trace and _CACHE.get("tmpdir"):
        kw["tmpdir"] = _CACHE["tmpdir"]
    res = run_bass_kernel_spmd(nc, in_maps, core_ids=list(range(NCORES)),
                               trace=trace, **kw)
    _CACHE["last_result"] = res
    rs = res.results

    embs = [
        np.concatenate([rs[i][f"emb{l}"].T for i in range(NCORES)], axis=0)
        for l in range(3)
    ]
    certs = [
        np.concatenate([rs[i][f"cert{l}"].T for i in range(NCORES)], axis=0)
        for l in range(3)
    ]
    links = [
        (
            np.sum(
                np.stack([rs[i][f"link{l}"] for i in range(NCORES)]).astype(np.float64),
                axis=0,
            )
            / B
        ).astype(np.float32)
        for l in range(2)
    ]
    return (*embs, *certs, *links)


# revision 14
# speedup vs baseline: 1.2457x; 1.0208x over previous
"""AVSL-Graph fused kernel for Trainium2 (8 NeuronCores, data-parallel over batch).

Computation (per reference):
  for each level l in {0,1,2}:
    cam_l[b,r,hw] = sum_c w_l[r,c] * fmap_l[b,c,hw]          (1x1-conv GEMM)
    emb_l[b,r]    = mean_hw(cam_l) + bias_l[r]               (== pooled @ w.T + b)
    cert_l[b,r]   = std_hw(cam_l, ddof=1)
  link_l = sum_b  norm(pool2x2(cam_l))[b] @ norm(cam_{l+1})[b].T / B

Sharding: batch B=64 split 8 ways (8 samples/core). Each core computes its
embedding/certainty rows and an unnormalized link partial (sum over its local
samples, accumulated in PSUM); the host sums the 8 partials and divides by B.

The CAM GEMMs run with float32r operands (4-byte fp32 storage, reduced-precision
multiplier array at 4x the fp32 matmul rate; measured ~1.5e-4 rel err) with fp32
PSUM accumulation. Everything downstream (stats, norms, links) stays fp32.
"""

import os
import sys

import numpy as np

for _p in ("/opt/trn_rl_repo", "/root/.axon_site/_ro/trn_rl_repo"):
    if os.path.isdir(_p) and _p not in sys.path:
        sys.path.append(_p)

B = 64
R = 128
NCORES = 8
BLOC = B // NCORES  # 8 samples per core
CH = (512, 1024, 2048)
HW = (784, 196, 49)  # 28*28, 14*14, 7*7
KC = (4, 8, 16)  # contraction chunks of 128 per level

_CACHE = {}


def _build():
    import concourse.bass as bass
    import concourse.bacc as bacc
    import concourse.tile as tile
    from concourse import masks, mybir

    f32 = mybir.dt.float32
    f32r = mybir.dt.float32r
    AX = mybir.AxisListType
    AF = mybir.ActivationFunctionType

    nc = bacc.Bacc("TRN2", target_bir_lowering=False, debug=False)

    fm0 = nc.dram_tensor("fmap0", (BLOC, CH[0], HW[0]), f32r, kind="ExternalInput")
    # fmap1/fmap2 arrive host-transposed to (C, B_loc, HW) so HBM reads are
    # contiguous (b, hw) runs per (chunk, partition)
    fm1 = nc.dram_tensor("fmap1", (CH[1], BLOC, HW[1]), f32r, kind="ExternalInput")
    fm2 = nc.dram_tensor("fmap2", (CH[2], BLOC, HW[2]), f32r, kind="ExternalInput")
    wT = [
        nc.dram_tensor(f"w{l}T", (CH[l], R), f32r, kind="ExternalInput")
        for l in range(3)
    ]
    bias = nc.dram_tensor("bias", (R, 3), f32, kind="ExternalInput")
    emb_d = [
        nc.dram_tensor(f"emb{l}", (R, BLOC), f32, kind="ExternalOutput")
        for l in range(3)
    ]
    cert_d = [
        nc.dram_tensor(f"cert{l}", (R, BLOC), f32, kind="ExternalOutput")
        for l in range(3)
    ]
    link_d = [
        nc.dram_tensor(f"link{l}", (R, R), f32, kind="ExternalOutput")
        for l in range(2)
    ]

    with tile.TileContext(nc) as tc:
        with (
            tc.tile_pool(name="const", bufs=1) as const,
            tc.tile_pool(name="f0p", bufs=3) as f0p,
            tc.tile_pool(name="f1p", bufs=2) as f1p,
            tc.tile_pool(name="f2p", bufs=1) as f2p,
            tc.tile_pool(name="work", bufs=4) as work,
            tc.tile_pool(name="stats", bufs=4) as stats,
            tc.tile_pool(name="outp", bufs=1) as outp,
            tc.tile_pool(name="ps_cam0", bufs=3, space="PSUM") as ps_cam0,
            tc.tile_pool(name="ps_cam1", bufs=1, space="PSUM") as ps_cam1,
            tc.tile_pool(name="ps_cam2", bufs=1, space="PSUM") as ps_cam2,
            tc.tile_pool(name="ps_link", bufs=1, space="PSUM") as ps_link,
            tc.tile_pool(name="ps_tr", bufs=2, space="PSUM") as ps_tr,
        ):
            # ---- constants ----
            wt = []
            for l in range(3):
                t = const.tile([128, KC[l], 128], f32r, tag=f"wt{l}", name=f"wt{l}")
                nc.sync.dma_start(
                    t[:], wT[l][:].rearrange("(k p) r -> p k r", p=128)
                )
                wt.append(t)
            bias_t = const.tile([128, 3], f32, tag="bias")
            nc.sync.dma_start(bias_t[:], bias[:])
            ident = const.tile([128, 128], f32, tag="ident")
            masks.make_identity(nc, ident[:])

            # ---- output accumulators in SBUF ----
            emb_t = [outp.tile([128, BLOC], f32, tag=f"emb{l}", name=f"emb_t{l}")
                     for l in range(3)]
            cert_t = [outp.tile([128, BLOC], f32, tag=f"cert{l}", name=f"cert_t{l}")
                      for l in range(3)]
            loT1s = outp.tile([128, BLOC, 128], f32, tag="loT1s")

            f2t = f2p.tile([128, KC[2], BLOC, HW[2]], f32r, tag="f2")
            fm2v = fm2[:].rearrange("(k p) b hw -> p k b hw", p=128)

            # link0/link1 share one PSUM bank: link1's accumulation group
            # starts only after link0's has stopped, so the bank-wide
            # has_written clear on its start=True cannot corrupt link0 data.
            link_ps = ps_link.tile([128, 2, 128], f32, tag="link")
            link0_t = link_ps[:, 0, :]
            link1_t = link_ps[:, 1, :]
            cam2 = ps_cam2.tile([128, BLOC, HW[2]], f32, tag="cam2")

            def stage1(b):
                """DMAs, GEMMs, evacuation, stats, normalized lo/hi for sample b."""
                nonlocal cam1sb
                if b % 2 == 0:
                    f1t = f1p.tile([128, KC[1], 2, HW[1]], f32r, tag="f1",
                                   name=f"f1t_{b}")
                    nc.sync.dma_start(
                        f1t[:],
                        fm1[:, b : b + 2, :].rearrange(
                            "(k p) b hw -> p k b hw", p=128
                        ),
                    )
                    cam1 = ps_cam1.tile([128, 2, HW[1]], f32, tag="cam1",
                                        name=f"cam1_{b}")
                    for k in range(KC[1]):
                        nc.tensor.matmul(
                            cam1[:], wt[1][:, k, :], f1t[:, k, :, :],
                            start=(k == 0), stop=(k == KC[1] - 1),
                        )
                f0t = f0p.tile([128, KC[0], HW[0]], f32r, tag="f0", name=f"f0t_{b}")
                nc.sync.dma_start(
                    f0t[:], fm0[b].rearrange("(k p) hw -> p k hw", p=128)
                )
                if b % 2 == 0:
                    g = b // 2
                    nc.sync.dma_start(
                        f2t[:, 4 * g : 4 * g + 4, :, :],
                        fm2v[:, 4 * g : 4 * g + 4, :, :],
                    )
                cam0h = [ps_cam0.tile([128, 512], f32, tag="cam0",
                                      name=f"cam0_{b}_{h}") for h in range(2)]
                for k in range(KC[0]):
                    for h in range(2):
                        nc.tensor.matmul(
                            cam0h[h][:, 0:392],
                            wt[0][:, k, :],
                            f0t[:, k, h * 392 : (h + 1) * 392],
                            start=(k == 0), stop=(k == KC[0] - 1),
                        )
                # cam2 contraction chunks, 2 per iteration, delayed 2 iterations
                # behind their DMA so the PE never waits on fmap2
                if b >= 2:
                    for k in (2 * b - 4, 2 * b - 3):
                        nc.tensor.matmul(
                            cam2[:], wt[2][:, k, :], f2t[:, k, :, :],
                            start=(k == 0), stop=(k == KC[2] - 1),
                        )

                # ---- evacuate PSUM fast (frees the banks for the next GEMM) ----
                cam0sb = work.tile([128, 2, 392], f32, tag="cam0sb",
                                   name=f"cam0sb_{b}")
                for h in range(2):
                    nc.vector.tensor_copy(cam0sb[:, h, :], cam0h[h][:, 0:392])
                if b % 2 == 0:
                    cam1sb = work.tile([128, 2, HW[1]], f32, tag="cam1sb",
                                       name=f"cam1sb_{b}")
                    nc.vector.tensor_copy(cam1sb[:], cam1[:])

                # ---- level 0 stats ----
                st0 = stats.tile([128, 2, 6], f32, tag="st0", name=f"st0_{b}")
                for h in range(2):
                    nc.vector.bn_stats(st0[:, h, :], cam0sb[:, h, :])
                mv0 = stats.tile([128, 2], f32, tag="mv0", name=f"mv0_{b}")
                nc.vector.bn_aggr(mv0[:], st0[:])
                nc.scalar.activation(
                    emb_t[0][:, b : b + 1], mv0[:, 0:1], AF.Identity,
                    bias=bias_t[:, 0:1], scale=1.0,
                )
                nc.scalar.activation(
                    cert_t[0][:, b : b + 1], mv0[:, 1:2], AF.Sqrt,
                    scale=float(HW[0]) / float(HW[0] - 1),
                )
                # ---- 2x2 sum-pool of cam0 on gpsimd (scale cancels in norm) ----
                praw0 = work.tile([128, 196], f32, tag="praw0", name=f"praw0_{b}")
                tp = work.tile([128, 2, 98], f32, tag="pool_tmp", name=f"ptmp_{b}")
                for h in range(2):
                    v = cam0sb[:, h, :].rearrange(
                        "p (oh dh ow dw) -> p oh ow dh dw", oh=7, dh=2, ow=14, dw=2
                    )
                    t1 = tp[:, 0, :].rearrange("p (oh ow) -> p oh ow", oh=7)
                    t2 = tp[:, 1, :].rearrange("p (oh ow) -> p oh ow", oh=7)
                    po = praw0[:, h * 98 : (h + 1) * 98].rearrange(
                        "p (oh ow) -> p oh ow", oh=7
                    )
                    nc.gpsimd.tensor_add(t1, v[:, :, :, 0, 0], v[:, :, :, 0, 1])
                    nc.gpsimd.tensor_add(t2, v[:, :, :, 1, 0], v[:, :, :, 1, 1])
                    nc.gpsimd.tensor_add(po, t1, t2)
                sq0 = work.tile([128, 196], f32, tag="sq0", name=f"sq0_{b}")
                q0 = stats.tile([128, 1], f32, tag="q0", name=f"q0_{b}")
                nc.scalar.activation(sq0[:], praw0[:], AF.Square, accum_out=q0[:])
                inv_lo0 = stats.tile([128, 1], f32, tag="inv_lo0", name=f"ilo0_{b}")
                nc.scalar.sqrt(inv_lo0[:], q0[:])
                nc.vector.reciprocal(inv_lo0[:], inv_lo0[:])
                lo0 = work.tile([128, 196], f32, tag="lo0", name=f"lo0_{b}")
                nc.vector.tensor_scalar_mul(lo0[:], praw0[:], inv_lo0[:])

                # ---- level 1 stats + hi1 + pooled lo1 ----
                camb = cam1sb[:, b % 2, :]
                st1 = stats.tile([128, 6], f32, tag="st1", name=f"st1_{b}")
                nc.vector.bn_stats(st1[:], camb)
                mv1 = stats.tile([128, 2], f32, tag="mv1", name=f"mv1_{b}")
                nc.vector.bn_aggr(mv1[:], st1[:])
                nc.scalar.activation(
                    emb_t[1][:, b : b + 1], mv1[:, 0:1], AF.Identity,
                    bias=bias_t[:, 1:2], scale=1.0,
                )
                nc.scalar.activation(
                    cert_t[1][:, b : b + 1], mv1[:, 1:2], AF.Sqrt,
                    scale=float(HW[1]) / float(HW[1] - 1),
                )
                nh1 = stats.tile([128, 1], f32, tag="nh1", name=f"nh1_{b}")
                nc.gpsimd.tensor_mul(nh1[:], mv1[:, 0:1], mv1[:, 0:1])
                nc.gpsimd.tensor_add(nh1[:], nh1[:], mv1[:, 1:2])
                nc.scalar.activation(nh1[:], nh1[:], AF.Sqrt, scale=float(HW[1]))
                nc.vector.reciprocal(nh1[:], nh1[:])
                hi1 = work.tile([128, 196], f32, tag="hi1", name=f"hi1_{b}")
                nc.vector.tensor_scalar_mul(hi1[:], camb, nh1[:])

                praw1 = work.tile([128, 49], f32, tag="praw1", name=f"praw1_{b}")
                v1 = camb.rearrange(
                    "p (oh dh ow dw) -> p oh ow dh dw", oh=7, dh=2, ow=7, dw=2
                )
                tq = work.tile([128, 2, 49], f32, tag="pool_tmp1", name=f"ptmp1_{b}")
                u1 = tq[:, 0, :].rearrange("p (oh ow) -> p oh ow", oh=7)
                u2 = tq[:, 1, :].rearrange("p (oh ow) -> p oh ow", oh=7)
                pv = praw1[:].rearrange("p (oh ow) -> p oh ow", oh=7)
                nc.gpsimd.tensor_add(u1, v1[:, :, :, 0, 0], v1[:, :, :, 0, 1])
                nc.gpsimd.tensor_add(u2, v1[:, :, :, 1, 0], v1[:, :, :, 1, 1])
                nc.gpsimd.tensor_add(pv, u1, u2)
                sq1 = work.tile([128, 49], f32, tag="sq1", name=f"sq1_{b}")
                q1 = stats.tile([128, 1], f32, tag="q1", name=f"q1_{b}")
                nc.scalar.activation(sq1[:], praw1[:], AF.Square, accum_out=q1[:])
                inv_lo1 = stats.tile([128, 1], f32, tag="inv_lo1", name=f"ilo1_{b}")
                nc.scalar.sqrt(inv_lo1[:], q1[:])
                nc.vector.reciprocal(inv_lo1[:], inv_lo1[:])
                lo1 = work.tile([128, 49], f32, tag="lo1", name=f"lo1_{b}")
                nc.vector.tensor_scalar_mul(lo1[:], praw1[:], inv_lo1[:])
                return lo0, hi1, lo1

            def stage2(b, lo0, hi1, lo1):
                """Transposes + link0 accumulation for sample b (runs 1 behind)."""
                tr0 = ps_tr.tile([128, 4, 128], f32, tag="tr", name=f"tr0_{b}")
                nc.tensor.transpose(tr0[:, 0, :], lo0[:, 0:128], ident[:])
                nc.tensor.transpose(tr0[0:68, 1, :], lo0[:, 128:196], ident[:])
                nc.tensor.transpose(tr0[:, 2, :], hi1[:, 0:128], ident[:])
                nc.tensor.transpose(tr0[0:68, 3, :], hi1[:, 128:196], ident[:])
                tr1 = ps_tr.tile([128, 4, 128], f32, tag="tr", name=f"tr1_{b}")
                nc.tensor.transpose(tr1[0:49, 0, :], lo1[:], ident[:])
                loHiT = work.tile([128, 4, 128], f32, tag="loHiT", name=f"loHiT_{b}")
                nc.vector.tensor_copy(loHiT[:], tr0[:])
                nc.vector.tensor_copy(loT1s[:, b, :], tr1[:, 0, :])
                nc.tensor.matmul(
                    link0_t, loHiT[:, 0, :], loHiT[:, 2, :],
                    start=(b == 0), stop=False,
                )
                nc.tensor.matmul(
                    link0_t, loHiT[0:68, 1, :], loHiT[0:68, 3, :],
                    start=False, stop=(b == BLOC - 1),
                )

            # ============ phase A: pipelined over samples ============
            cam1sb = None
            staged = {}
            for b in range(BLOC + 2):
                if b >= 2:
                    stage2(b - 2, *staged.pop(b - 2))
                if b < BLOC:
                    staged[b] = stage1(b)

            # last cam2 contraction chunks
            for k in (KC[2] - 4, KC[2] - 3, KC[2] - 2, KC[2] - 1):
                nc.tensor.matmul(
                    cam2[:], wt[2][:, k, :], f2t[:, k, :, :],
                    start=(k == 0), stop=(k == KC[2] - 1),
                )

            # ============ phase B: level 2 stats + link1 (batched) ============
            sqa = work.tile([128, BLOC, HW[2]], f32, tag="sqa")
            nc.scalar.activation(sqa[:], cam2[:], AF.Square)
            S2 = stats.tile([128, BLOC], f32, tag="S2")
            nc.vector.reduce_sum(S2[:], cam2[:], axis=AX.X)
            Q2 = stats.tile([128, BLOC], f32, tag="Q2")
            nc.vector.reduce_sum(Q2[:], sqa[:], axis=AX.X)
            nc.vector.tensor_scalar(
                emb_t[2][:], S2[:], 1.0 / HW[2], bias_t[:, 2:3],
                op0=mybir.AluOpType.mult, op1=mybir.AluOpType.add,
            )
            v2a = stats.tile([128, BLOC], f32, tag="v2a")
            nc.gpsimd.tensor_mul(v2a[:], S2[:], S2[:])
            nc.gpsimd.tensor_scalar_mul(v2a[:], v2a[:], -1.0 / HW[2])
            nc.gpsimd.tensor_add(v2a[:], v2a[:], Q2[:])
            nc.scalar.activation(
                cert_t[2][:], v2a[:], AF.Sqrt, scale=1.0 / (HW[2] - 1)
            )
            nh2a = stats.tile([128, BLOC], f32, tag="nh2a")
            nc.scalar.sqrt(nh2a[:], Q2[:])
            nc.vector.reciprocal(nh2a[:], nh2a[:])
            hi2a = work.tile([128, BLOC, HW[2]], f32, tag="hi2a")
            nh2v = nh2a[:]
            nh2b = bass.AP(
                tensor=nh2v.tensor, offset=nh2v.offset,
                ap=[nh2v.ap[0], nh2v.ap[1], [0, HW[2]]],
            )
            nc.vector.tensor_mul(hi2a[:], cam2[:], nh2b)

            hiT2s = []
            for g in range(2):
                trg = ps_tr.tile([128, 4, 128], f32, tag="tr", name=f"tr2_{g}")
                for j in range(4):
                    nc.tensor.transpose(
                        trg[0:49, j, :], hi2a[:, 4 * g + j, :], ident[:]
                    )
                ht = work.tile([128, 4, 128], f32, tag="hiT2s", name=f"hiT2s{g}")
                nc.vector.tensor_copy(ht[:], trg[:])
                hiT2s.append(ht)
            for b in range(BLOC):
                nc.tensor.matmul(
                    link1_t, loT1s[0:49, b, :], hiT2s[b // 4][0:49, b % 4, :],
                    start=(b == 0), stop=(b == BLOC - 1),
                )

            # ---- write outputs ----
            link_sb = outp.tile([128, 2, 128], f32, tag="link_sb")
            nc.vector.tensor_copy(link_sb[:], link_ps[:])
            nc.sync.dma_start(link_d[0][:], link_sb[:, 0, :])
            nc.sync.dma_start(link_d[1][:], link_sb[:, 1, :])
            for l in range(3):
                nc.sync.dma_start(emb_d[l][:], emb_t[l][:])
                nc.sync.dma_start(cert_d[l][:], cert_t[l][:])

    nc.compile()
    return nc


def _get_nc():
    if "nc" not in _CACHE:
        _CACHE["nc"] = _build()
    return _CACHE["nc"]


def kernel(**inputs):
    from concourse.bass_utils import run_bass_kernel_spmd

    fmaps = [np.asarray(inputs[f"fmap{l}"], dtype=np.float32) for l in range(3)]
    ws = [np.asarray(inputs[f"w{l}"], dtype=np.float32) for l in range(3)]
    bs = [np.asarray(inputs[f"b{l}"], dtype=np.float32) for l in range(3)]

    wTs = [np.ascontiguousarray(w.T) for w in ws]
    bias = np.ascontiguousarray(np.stack(bs, axis=1))  # (R, 3)

    f0 = np.ascontiguousarray(fmaps[0].reshape(B, CH[0], HW[0]))
    # fmap1/fmap2 shipped channel-major per core: (C, B_loc, HW)
    f1 = np.ascontiguousarray(
        fmaps[1].reshape(B, CH[1], HW[1]).transpose(1, 0, 2)
    )
    f2 = np.ascontiguousarray(
        fmaps[2].reshape(B, CH[2], HW[2]).transpose(1, 0, 2)
    )

    nc = _get_nc()
    in_maps = []
    for i in range(NCORES):
        sl = slice(i * BLOC, (i + 1) * BLOC)
        m = {
            "bias": bias,
            "w0T": wTs[0], "w1T": wTs[1], "w2T": wTs[2],
            "fmap0": f0[sl],
            "fmap1": np.ascontiguousarray(f1[:, sl, :]),
            "fmap2": np.ascontiguousarray(f2[:, sl, :]),
        }
        in_maps.append(m)

    trace = os.environ.get("BASS_KERNEL_TRACE") == "1"
    kw = {}
    if trace and _CACHE.get("tmpdir"):
        kw["tmpdir"] = _CACHE["tmpdir"]
    res = run_bass_kernel_spmd(nc, in_maps, core_ids=list(range(NCORES)),
                               trace=trace, **kw)
    _CACHE["last_result"] = res
    rs = res.results

    embs = [
        np.concatenate([rs[i][f"emb{l}"].T for i in range(NCORES)], axis=0)
        for l in range(3)
    ]
    certs = [
        np.concatenate([rs[i][f"cert{l}"].T for i in range(NCORES)], axis=0)
        for l in range(3)
    ]
    links = [
        (
            np.sum(
                np.stack([rs[i][f"link{l}"] for i in range(NCORES)]).astype(np.float64),
                axis=0,
            )
            / B
        ).astype(np.float32)
        for l in range(2)
    ]
    return (*embs, *certs, *links)


# revision 15
# speedup vs baseline: 1.3740x; 1.1030x over previous
"""AVSL-Graph fused kernel for Trainium2 (8 NeuronCores, data-parallel over batch).

Computation (per reference):
  for each level l in {0,1,2}:
    cam_l[b,r,hw] = sum_c w_l[r,c] * fmap_l[b,c,hw]          (1x1-conv GEMM)
    emb_l[b,r]    = mean_hw(cam_l) + bias_l[r]               (== pooled @ w.T + b)
    cert_l[b,r]   = std_hw(cam_l, ddof=1)
  link_l = sum_b  norm(pool2x2(cam_l))[b] @ norm(cam_{l+1})[b].T / B

Sharding: batch B=64 split 8 ways (8 samples/core). Each core computes its
embedding/certainty rows and an unnormalized link partial (sum over its local
samples, accumulated in PSUM); the host sums the 8 partials and divides by B.

The CAM GEMMs run with float32r operands (4-byte fp32 storage, reduced-precision
multiplier array at 4x the fp32 matmul rate; measured ~1.5e-4 rel err) with fp32
PSUM accumulation. Everything downstream (stats, norms, links) stays fp32.
"""

import os
import sys

import numpy as np

for _p in ("/opt/trn_rl_repo", "/root/.axon_site/_ro/trn_rl_repo"):
    if os.path.isdir(_p) and _p not in sys.path:
        sys.path.append(_p)

B = 64
R = 128
NCORES = 8
BLOC = B // NCORES  # 8 samples per core
CH = (512, 1024, 2048)
HW = (784, 196, 49)  # 28*28, 14*14, 7*7
KC = (4, 8, 16)  # contraction chunks of 128 per level

_CACHE = {}


def _build():
    import concourse.bass as bass
    import concourse.bacc as bacc
    import concourse.tile as tile
    from concourse import masks, mybir

    f32 = mybir.dt.float32
    f32r = mybir.dt.float32r
    AX = mybir.AxisListType
    AF = mybir.ActivationFunctionType

    nc = bacc.Bacc("TRN2", target_bir_lowering=False, debug=False)

    fm0 = nc.dram_tensor("fmap0", (BLOC, CH[0], HW[0]), f32r, kind="ExternalInput")
    # fmap1/fmap2 arrive host-transposed to (C, B_loc, HW) so HBM reads are
    # contiguous (b, hw) runs per (chunk, partition)
    fm1 = nc.dram_tensor("fmap1", (CH[1], BLOC, HW[1]), f32r, kind="ExternalInput")
    fm2 = nc.dram_tensor("fmap2", (CH[2], BLOC, HW[2]), f32r, kind="ExternalInput")
    wT = [
        nc.dram_tensor(f"w{l}T", (CH[l], R), f32r, kind="ExternalInput")
        for l in range(3)
    ]
    bias = nc.dram_tensor("bias", (R, 3), f32, kind="ExternalInput")
    emb_d = [
        nc.dram_tensor(f"emb{l}", (R, BLOC), f32, kind="ExternalOutput")
        for l in range(3)
    ]
    cert_d = [
        nc.dram_tensor(f"cert{l}", (R, BLOC), f32, kind="ExternalOutput")
        for l in range(3)
    ]
    link_d = [
        nc.dram_tensor(f"link{l}", (R, R), f32, kind="ExternalOutput")
        for l in range(2)
    ]

    with tile.TileContext(nc) as tc:
        with (
            tc.tile_pool(name="const", bufs=1) as const,
            tc.tile_pool(name="f0p", bufs=3) as f0p,
            tc.tile_pool(name="f1p", bufs=2) as f1p,
            tc.tile_pool(name="f2p", bufs=1) as f2p,
            tc.tile_pool(name="work", bufs=4) as work,
            tc.tile_pool(name="stats", bufs=4) as stats,
            tc.tile_pool(name="outp", bufs=1) as outp,
            tc.tile_pool(name="ps_cam0", bufs=3, space="PSUM") as ps_cam0,
            tc.tile_pool(name="ps_cam1", bufs=1, space="PSUM") as ps_cam1,
            tc.tile_pool(name="ps_cam2", bufs=1, space="PSUM") as ps_cam2,
            tc.tile_pool(name="ps_link", bufs=1, space="PSUM") as ps_link,
            tc.tile_pool(name="ps_tr", bufs=2, space="PSUM") as ps_tr,
        ):
            # ---- constants ----
            wt = []
            for l in range(3):
                t = const.tile([128, KC[l], 128], f32r, tag=f"wt{l}", name=f"wt{l}")
                nc.sync.dma_start(
                    t[:], wT[l][:].rearrange("(k p) r -> p k r", p=128)
                )
                wt.append(t)
            bias_t = const.tile([128, 3], f32, tag="bias")
            nc.sync.dma_start(bias_t[:], bias[:])
            ident = const.tile([128, 128], f32, tag="ident")
            masks.make_identity(nc, ident[:])

            # ---- persistent accumulators in SBUF ----
            emb_t = [outp.tile([128, BLOC], f32, tag=f"emb{l}", name=f"emb_t{l}")
                     for l in range(3)]
            cert_t = [outp.tile([128, BLOC], f32, tag=f"cert{l}", name=f"cert_t{l}")
                      for l in range(3)]
            loT1s = outp.tile([128, BLOC, 128], f32, tag="loT1s")
            mvs0 = outp.tile([128, BLOC, 2], f32, tag="mvs0")  # mean/var cam0
            mvs1 = outp.tile([128, BLOC, 2], f32, tag="mvs1")  # mean/var cam1

            f2t = f2p.tile([128, KC[2], BLOC, HW[2]], f32r, tag="f2")
            fm2v = fm2[:].rearrange("(k p) b hw -> p k b hw", p=128)

            # link0/link1 share one PSUM bank: link1's accumulation group
            # starts only after link0's stopped, so its bank-wide has_written
            # clear cannot corrupt link0's data.
            link_ps = ps_link.tile([128, 2, 128], f32, tag="link")
            cam2 = ps_cam2.tile([128, BLOC, HW[2]], f32, tag="cam2")

            def stage_gemm(b):
                """DMAs + all GEMMs for sample b (PE-dense, no cross-engine deps)."""
                nonlocal cam1
                if b % 2 == 0:
                    f1t = f1p.tile([128, KC[1], 2, HW[1]], f32r, tag="f1",
                                   name=f"f1t_{b}")
                    nc.sync.dma_start(
                        f1t[:],
                        fm1[:, b : b + 2, :].rearrange(
                            "(k p) b hw -> p k b hw", p=128
                        ),
                    )
                f0t = f0p.tile([128, KC[0], HW[0]], f32r, tag="f0", name=f"f0t_{b}")
                nc.sync.dma_start(
                    f0t[:], fm0[b].rearrange("(k p) hw -> p k hw", p=128)
                )
                if b % 2 == 0:
                    g = b // 2
                    nc.sync.dma_start(
                        f2t[:, 4 * g : 4 * g + 4, :, :],
                        fm2v[:, 4 * g : 4 * g + 4, :, :],
                    )
                if b % 2 == 0:
                    cam1 = ps_cam1.tile([128, 2, HW[1]], f32, tag="cam1",
                                        name=f"cam1_{b}")
                    for k in range(KC[1]):
                        nc.tensor.matmul(
                            cam1[:], wt[1][:, k, :], f1t[:, k, :, :],
                            start=(k == 0), stop=(k == KC[1] - 1),
                        )
                cam0h = [ps_cam0.tile([128, 512], f32, tag="cam0",
                                      name=f"cam0_{b}_{h}") for h in range(2)]
                for k in range(KC[0]):
                    for h in range(2):
                        nc.tensor.matmul(
                            cam0h[h][:, 0:392],
                            wt[0][:, k, :],
                            f0t[:, k, h * 392 : (h + 1) * 392],
                            start=(k == 0), stop=(k == KC[0] - 1),
                        )
                if b >= 2:
                    for k in (2 * b - 4, 2 * b - 3):
                        nc.tensor.matmul(
                            cam2[:], wt[2][:, k, :], f2t[:, k, :, :],
                            start=(k == 0), stop=(k == KC[2] - 1),
                        )
                return cam0h

            def stage_stats(b, cam0h):
                """Evacuate PSUM, stats, pooled/normalized lo+hi for sample b."""
                nonlocal cam1sb
                cam0sb = work.tile([128, 2, 392], f32, tag="cam0sb",
                                   name=f"cam0sb_{b}")
                for h in range(2):
                    nc.vector.tensor_copy(cam0sb[:, h, :], cam0h[h][:, 0:392])
                if b % 2 == 0:
                    cam1sb = work.tile([128, 2, HW[1]], f32, tag="cam1sb",
                                       name=f"cam1sb_{b}")
                    nc.vector.tensor_copy(cam1sb[:], cam1[:])
                camb = cam1sb[:, b % 2, :]

                # packed norm^2 column: [q_lo0, q_lo1, n1*(mean^2+var)]
                qall = stats.tile([128, 3], f32, tag="qall", name=f"qall_{b}")

                # level-0 pooled lo (gpsimd) + bn stats (DVE)
                praw0 = work.tile([128, 196], f32, tag="praw0", name=f"praw0_{b}")
                tp = work.tile([128, 2, 98], f32, tag="pool_tmp", name=f"ptmp_{b}")
                for h in range(2):
                    v = cam0sb[:, h, :].rearrange(
                        "p (oh dh ow dw) -> p oh ow dh dw", oh=7, dh=2, ow=14, dw=2
                    )
                    t1 = tp[:, 0, :].rearrange("p (oh ow) -> p oh ow", oh=7)
                    t2 = tp[:, 1, :].rearrange("p (oh ow) -> p oh ow", oh=7)
                    po = praw0[:, h * 98 : (h + 1) * 98].rearrange(
                        "p (oh ow) -> p oh ow", oh=7
                    )
                    nc.gpsimd.tensor_add(t1, v[:, :, :, 0, 0], v[:, :, :, 0, 1])
                    nc.gpsimd.tensor_add(t2, v[:, :, :, 1, 0], v[:, :, :, 1, 1])
                    nc.gpsimd.tensor_add(po, t1, t2)
                st0 = stats.tile([128, 2, 6], f32, tag="st0", name=f"st0_{b}")
                for h in range(2):
                    nc.vector.bn_stats(st0[:, h, :], cam0sb[:, h, :])
                nc.vector.bn_aggr(mvs0[:, b, :], st0[:])

                # level-1 stats + pooled lo1 (DVE reduce)
                st1 = stats.tile([128, 6], f32, tag="st1", name=f"st1_{b}")
                nc.vector.bn_stats(st1[:], camb)
                nc.vector.bn_aggr(mvs1[:, b, :], st1[:])
                praw1 = work.tile([128, 49], f32, tag="praw1", name=f"praw1_{b}")
                nc.vector.reduce_sum(
                    praw1[:].rearrange("p (oh ow) -> p oh ow", oh=7),
                    camb.rearrange(
                        "p (oh dh ow dw) -> p oh ow dh dw", oh=7, dh=2, ow=7, dw=2
                    ),
                    axis=AX.XY,
                )
                # ||cam1||^2 = n*(mean^2+var) on gpsimd
                nt = stats.tile([128, 1], f32, tag="nt", name=f"nt_{b}")
                nc.gpsimd.tensor_mul(nt[:], mvs1[:, b, 0:1], mvs1[:, b, 0:1])
                nc.gpsimd.tensor_add(nt[:], nt[:], mvs1[:, b, 1:2])
                nc.gpsimd.tensor_scalar_mul(qall[:, 2:3], nt[:], float(HW[1]))

                # squared sums of the pooled maps (ACT, accumulating)
                sq0 = work.tile([128, 196], f32, tag="sq0", name=f"sq0_{b}")
                nc.scalar.activation(sq0[:], praw0[:], AF.Square,
                                     accum_out=qall[:, 0:1])
                sq1 = work.tile([128, 49], f32, tag="sq1", name=f"sq1_{b}")
                nc.scalar.activation(sq1[:], praw1[:], AF.Square,
                                     accum_out=qall[:, 1:2])

                # one sqrt + one reciprocal for all three norms
                inva = stats.tile([128, 3], f32, tag="inva", name=f"inva_{b}")
                nc.scalar.sqrt(inva[:], qall[:])
                nc.vector.reciprocal(inva[:], inva[:])

                lo0 = work.tile([128, 196], f32, tag="lo0", name=f"lo0_{b}")
                nc.vector.tensor_scalar_mul(lo0[:], praw0[:], inva[:, 0:1])
                lo1 = work.tile([128, 49], f32, tag="lo1", name=f"lo1_{b}")
                nc.scalar.mul(lo1[:], praw1[:], inva[:, 1:2])
                hi1 = work.tile([128, 196], f32, tag="hi1", name=f"hi1_{b}")
                nc.scalar.mul(hi1[:], camb, inva[:, 2:3])
                return lo0, hi1, lo1

            def stage_tr(b, lo0, hi1, lo1):
                tr0 = ps_tr.tile([128, 4, 128], f32, tag="tr", name=f"tr0_{b}")
                nc.tensor.transpose(tr0[:, 0, :], lo0[:, 0:128], ident[:])
                nc.tensor.transpose(tr0[0:68, 1, :], lo0[:, 128:196], ident[:])
                nc.tensor.transpose(tr0[:, 2, :], hi1[:, 0:128], ident[:])
                nc.tensor.transpose(tr0[0:68, 3, :], hi1[:, 128:196], ident[:])
                tr1 = ps_tr.tile([128, 4, 128], f32, tag="tr", name=f"tr1_{b}")
                nc.tensor.transpose(tr1[0:49, 0, :], lo1[:], ident[:])
                loHiT = work.tile([128, 4, 128], f32, tag="loHiT", name=f"loHiT_{b}")
                nc.vector.tensor_copy(loHiT[:], tr0[:])
                nc.vector.tensor_copy(loT1s[:, b, :], tr1[:, 0, :])
                return loHiT

            def stage_link(b, loHiT):
                nc.tensor.matmul(
                    link_ps[:, 0, :], loHiT[:, 0, :], loHiT[:, 2, :],
                    start=(b == 0), stop=False,
                )
                nc.tensor.matmul(
                    link_ps[:, 0, :], loHiT[0:68, 1, :], loHiT[0:68, 3, :],
                    start=False, stop=(b == BLOC - 1),
                )

            # ============ phase A: 3-deep pipelined sample loop ============
            cam1 = None
            cam1sb = None
            gemm_out = {}
            norm_out = {}
            tr_out = {}
            for i in range(BLOC + 2):
                if i < BLOC:
                    gemm_out[i] = stage_gemm(i)
                if 1 <= i <= BLOC:
                    tr_out[i - 1] = stage_tr(i - 1, *norm_out.pop(i - 1))
                if 2 <= i:
                    stage_link(i - 2, tr_out.pop(i - 2))
                if i < BLOC:
                    norm_out[i] = stage_stats(i, gemm_out.pop(i))

            # last cam2 contraction chunks
            for k in (KC[2] - 4, KC[2] - 3, KC[2] - 2, KC[2] - 1):
                nc.tensor.matmul(
                    cam2[:], wt[2][:, k, :], f2t[:, k, :, :],
                    start=(k == 0), stop=(k == KC[2] - 1),
                )

            # ---- batched emb/cert for levels 0/1 ----
            for l, mvs in ((0, mvs0), (1, mvs1)):
                nc.vector.tensor_scalar(
                    emb_t[l][:], mvs[:, :, 0], 1.0, bias_t[:, l : l + 1],
                    op0=mybir.AluOpType.mult, op1=mybir.AluOpType.add,
                )
                nc.scalar.activation(
                    cert_t[l][:], mvs[:, :, 1], AF.Sqrt,
                    scale=float(HW[l]) / float(HW[l] - 1),
                )

            # ============ phase B: level 2 stats + link1 (batched) ============
            sqa = work.tile([128, BLOC, HW[2]], f32, tag="sqa")
            nc.scalar.activation(sqa[:], cam2[:], AF.Square)
            S2 = stats.tile([128, BLOC], f32, tag="S2")
            nc.vector.reduce_sum(S2[:], cam2[:], axis=AX.X)
            Q2 = stats.tile([128, BLOC], f32, tag="Q2")
            nc.vector.reduce_sum(Q2[:], sqa[:], axis=AX.X)
            nc.vector.tensor_scalar(
                emb_t[2][:], S2[:], 1.0 / HW[2], bias_t[:, 2:3],
                op0=mybir.AluOpType.mult, op1=mybir.AluOpType.add,
            )
            v2a = stats.tile([128, BLOC], f32, tag="v2a")
            nc.gpsimd.tensor_mul(v2a[:], S2[:], S2[:])
            nc.gpsimd.tensor_scalar_mul(v2a[:], v2a[:], -1.0 / HW[2])
            nc.gpsimd.tensor_add(v2a[:], v2a[:], Q2[:])
            nc.scalar.activation(
                cert_t[2][:], v2a[:], AF.Sqrt, scale=1.0 / (HW[2] - 1)
            )
            nh2a = stats.tile([128, BLOC], f32, tag="nh2a")
            nc.scalar.sqrt(nh2a[:], Q2[:])
            nc.vector.reciprocal(nh2a[:], nh2a[:])
            hi2a = work.tile([128, BLOC, HW[2]], f32, tag="hi2a")
            nh2v = nh2a[:]
            nh2b = bass.AP(
                tensor=nh2v.tensor, offset=nh2v.offset,
                ap=[nh2v.ap[0], nh2v.ap[1], [0, HW[2]]],
            )
            nc.vector.tensor_mul(hi2a[:], cam2[:], nh2b)

            hiT2s = []
            for g in range(2):
                trg = ps_tr.tile([128, 4, 128], f32, tag="tr", name=f"tr2_{g}")
                for j in range(4):
                    nc.tensor.transpose(
                        trg[0:49, j, :], hi2a[:, 4 * g + j, :], ident[:]
                    )
                ht = work.tile([128, 4, 128], f32, tag="hiT2s", name=f"hiT2s{g}")
                nc.vector.tensor_copy(ht[:], trg[:])
                hiT2s.append(ht)
            for b in range(BLOC):
                nc.tensor.matmul(
                    link_ps[:, 1, :], loT1s[0:49, b, :],
                    hiT2s[b // 4][0:49, b % 4, :],
                    start=(b == 0), stop=(b == BLOC - 1),
                )

            # ---- write outputs ----
            link_sb = outp.tile([128, 2, 128], f32, tag="link_sb")
            nc.vector.tensor_copy(link_sb[:], link_ps[:])
            nc.sync.dma_start(link_d[0][:], link_sb[:, 0, :])
            nc.sync.dma_start(link_d[1][:], link_sb[:, 1, :])
            for l in range(3):
                nc.sync.dma_start(emb_d[l][:], emb_t[l][:])
                nc.sync.dma_start(cert_d[l][:], cert_t[l][:])

    nc.compile()
    return nc


def _get_nc():
    if "nc" not in _CACHE:
        _CACHE["nc"] = _build()
    return _CACHE["nc"]


def kernel(**inputs):
    from concourse.bass_utils import run_bass_kernel_spmd

    fmaps = [np.asarray(inputs[f"fmap{l}"], dtype=np.float32) for l in range(3)]
    ws = [np.asarray(inputs[f"w{l}"], dtype=np.float32) for l in range(3)]
    bs = [np.asarray(inputs[f"b{l}"], dtype=np.float32) for l in range(3)]

    wTs = [np.ascontiguousarray(w.T) for w in ws]
    bias = np.ascontiguousarray(np.stack(bs, axis=1))  # (R, 3)

    f0 = np.ascontiguousarray(fmaps[0].reshape(B, CH[0], HW[0]))
    # fmap1/fmap2 shipped channel-major per core: (C, B_loc, HW)
    f1 = np.ascontiguousarray(
        fmaps[1].reshape(B, CH[1], HW[1]).transpose(1, 0, 2)
    )
    f2 = np.ascontiguousarray(
        fmaps[2].reshape(B, CH[2], HW[2]).transpose(1, 0, 2)
    )

    nc = _get_nc()
    in_maps = []
    for i in range(NCORES):
        sl = slice(i * BLOC, (i + 1) * BLOC)
        m = {
            "bias": bias,
            "w0T": wTs[0], "w1T": wTs[1], "w2T": wTs[2],
            "fmap0": f0[sl],
            "fmap1": np.ascontiguousarray(f1[:, sl, :]),
            "fmap2": np.ascontiguousarray(f2[:, sl, :]),
        }
        in_maps.append(m)

    trace = os.environ.get("BASS_KERNEL_TRACE") == "1"
    kw = {}
    if trace and _CACHE.get("tmpdir"):
        kw["tmpdir"] = _CACHE["tmpdir"]
    res = run_bass_kernel_spmd(nc, in_maps, core_ids=list(range(NCORES)),
                               trace=trace, **kw)
    _CACHE["last_result"] = res
    rs = res.results

    embs = [
        np.concatenate([rs[i][f"emb{l}"].T for i in range(NCORES)], axis=0)
        for l in range(3)
    ]
    certs = [
        np.concatenate([rs[i][f"cert{l}"].T for i in range(NCORES)], axis=0)
        for l in range(3)
    ]
    links = [
        (
            np.sum(
                np.stack([rs[i][f"link{l}"] for i in range(NCORES)]).astype(np.float64),
                axis=0,
            )
            / B
        ).astype(np.float32)
        for l in range(2)
    ]
    return (*embs, *certs, *links)
